# revision 9
# baseline (speedup 1.0000x reference)
"""nn_GT_7327214207519 — 2-layer TransformerConv GNN (heads=4) on 8 trn2 NeuronCores.

Strategy (edge/dst-parallel):
  * Nodes are split into 8 contiguous ranges (2500/core). Each core owns the
    destination-side softmax + aggregation for its node range, so no
    cross-core reduction is needed for the attention itself.
  * Per core, dst nodes are grouped into NB blocks of 128. Each block's edges
    (padded to T*128 slots) are fetched with one dma_gather of interleaved
    [K|V] rows; logits, exp and the weighted message sums are computed per
    128-edge tile, with a dst-match matrix M (via is_equal against an iota
    row) so TensorE matmuls accumulate per-destination sums in PSUM.
  * Softmax max-subtraction is skipped (logits are O(1) for this model), and
    biases are folded: bq into the q table, bk cancels in softmax, bv/bs into
    per-block constant rows.
  * Layer-1 k/v tables are computed replicated on every core from an
    AllGather of the transposed hidden h (bf16, chunked, overlapped with the
    layer-0 edge phase).
"""

import math
import numpy as np
import ml_dtypes

BF = ml_dtypes.bfloat16

# Problem constants (fixed by the task; kernel.py must be self-contained).
N_NODES, N_EDGES, D_IN, HID, OUT_CH, H = 20000, 320000, 128, 128, 128, 4
C = 128            # per-head channels, both layers
D = H * C          # 512
RANKS = 8

FULL_CFG = dict(N=N_NODES, RANKS=RANKS, NB=20, T=17, AGC=4)


def derive(cfg):
    g = dict(cfg)
    g["PER"] = g["N"] // g["RANKS"]          # real nodes per rank
    g["PERP"] = g["NB"] * 128                # padded nodes per rank
    assert g["PERP"] >= g["PER"]
    g["NI"] = g["T"] * 128                   # edge slots per block
    g["NTAB"] = g["RANKS"] * g["PERP"]       # padded table rows
    assert g["NB"] % g["AGC"] == 0
    g["AGB"] = g["NB"] // g["AGC"]           # blocks per allgather chunk
    g["AGW"] = g["AGB"] * 128                # nodes per allgather chunk
    assert g["NTAB"] < 32768                 # int16 gather indices
    return g


# ----------------------------------------------------------------------------
# Program builder
# ----------------------------------------------------------------------------

def build_program(cfg):
    import concourse.bass as bass
    import concourse.mybir as mybir
    import concourse.tile as tile
    from concourse import bacc
    from concourse.masks import make_identity

    g = derive(cfg)
    NB, T, NI, NTAB, PERP, AGC, AGB, AGW = (
        g["NB"], g["T"], g["NI"], g["NTAB"], g["PERP"], g["AGC"], g["AGB"], g["AGW"])
    NRANKS = g["RANKS"]
    F32, BF16, I16 = mybir.dt.float32, mybir.dt.bfloat16, mybir.dt.int16
    AF = mybir.ActivationFunctionType
    OP = mybir.AluOpType
    SCALE = 1.0 / math.sqrt(C)

    nc = bacc.Bacc("TRN2", target_bir_lowering=False, debug=False,
                   num_devices=NRANKS)

    # ---- I/O ----
    xT = nc.dram_tensor("xT", [D_IN, NTAB], BF16, kind="ExternalInput").ap()
    xT_own = nc.dram_tensor("xT_own", [D_IN, PERP], BF16, kind="ExternalInput").ap()
    w0kv = nc.dram_tensor("w0kv", [D_IN, 2 * D], BF16, kind="ExternalInput").ap()
    w0q = nc.dram_tensor("w0q", [D_IN, D], BF16, kind="ExternalInput").ap()
    w0s = nc.dram_tensor("w0s", [D_IN, D], BF16, kind="ExternalInput").ap()
    # layer-1 weights pre-chunked: [128, 4, out] where [:, sg, :] = W[sg*128:(sg+1)*128, :]
    w1kv = nc.dram_tensor("w1kv", [C, 4, 2 * D], BF16, kind="ExternalInput").ap()
    w1q = nc.dram_tensor("w1q", [C, 4, D], BF16, kind="ExternalInput").ap()
    w1s = nc.dram_tensor("w1s", [C, 4, OUT_CH], BF16, kind="ExternalInput").ap()
    b0q = nc.dram_tensor("b0q", [128, D], F32, kind="ExternalInput").ap()
    c0 = nc.dram_tensor("c0", [128, D], F32, kind="ExternalInput").ap()
    b1q = nc.dram_tensor("b1q", [128, D], F32, kind="ExternalInput").ap()
    c1 = nc.dram_tensor("c1", [128, OUT_CH], F32, kind="ExternalInput").ap()
    iota_in = nc.dram_tensor("iota", [128, 128], BF16, kind="ExternalInput").ap()
    kvidx = nc.dram_tensor("kvidx", [NB, 128, NI // 16], I16, kind="ExternalInput").ap()
    qidx = nc.dram_tensor("qidx", [NB, 128, NI // 16], I16, kind="ExternalInput").ap()
    dstrel = nc.dram_tensor("dstrel", [NB, 128, T], BF16, kind="ExternalInput").ap()
    out_t = nc.dram_tensor("out", [PERP, OUT_CH], F32, kind="ExternalOutput").ap()

    rg = [list(range(NRANKS))]

    with tile.TileContext(nc) as tc:
        with (
            tc.tile_pool(name="dram", bufs=1, space="DRAM") as dpool,
            tc.tile_pool(name="const", bufs=1) as cpool,
        ):
            kv0_t = dpool.tile([NTAB, 2 * D], BF16, tag="kv0")
            q0_t = dpool.tile([PERP, D], BF16, tag="q0")
            sk0_t = dpool.tile([PERP, D], F32, tag="sk0")
            kv1_t = dpool.tile([NTAB, 2 * D], BF16, tag="kv1")
            q1_t = dpool.tile([PERP, D], BF16, tag="q1")
            hT_own = [dpool.tile([D, AGW], BF16, tag=f"hTo{c}", name=f"hTo{c}")
                      for c in range(AGC)]
            hT_ag = [dpool.tile([NRANKS, D, AGW], BF16, tag=f"hTa{c}", name=f"hTa{c}",
                                addr_space="Shared")
                     for c in range(AGC)]

            # resident constants
            w0kv_s = cpool.tile([128, 2 * D], BF16, tag="w0kv")
            nc.sync.dma_start(out=w0kv_s[:], in_=w0kv)
            w0q_s = cpool.tile([128, D], BF16, tag="w0q")
            nc.sync.dma_start(out=w0q_s[:], in_=w0q)
            w0s_s = cpool.tile([128, D], BF16, tag="w0s")
            nc.sync.dma_start(out=w0s_s[:], in_=w0s)
            w1kv_s = cpool.tile([128, 4, 2 * D], BF16, tag="w1kv")
            nc.sync.dma_start(out=w1kv_s[:], in_=w1kv)
            w1q_s = cpool.tile([128, 4, D], BF16, tag="w1q")
            nc.sync.dma_start(out=w1q_s[:], in_=w1q)
            w1s_s = cpool.tile([128, 4, OUT_CH], BF16, tag="w1s")
            nc.sync.dma_start(out=w1s_s[:], in_=w1s)
            b0q_s = cpool.tile([128, D], F32, tag="b0q")
            nc.sync.dma_start(out=b0q_s[:], in_=b0q)
            c0_s = cpool.tile([128, D], F32, tag="c0")
            nc.sync.dma_start(out=c0_s[:], in_=c0)
            b1q_s = cpool.tile([128, D], F32, tag="b1q")
            nc.sync.dma_start(out=b1q_s[:], in_=b1q)
            c1_s = cpool.tile([128, OUT_CH], F32, tag="c1")
            nc.sync.dma_start(out=c1_s[:], in_=c1)
            iota_s = cpool.tile([128, 128], BF16, tag="iota")
            nc.sync.dma_start(out=iota_s[:], in_=iota_in)
            ident_s = cpool.tile([128, 128], BF16, tag="ident")
            make_identity(nc, ident_s[:])

            with (
                tc.tile_pool(name="work", bufs=2) as pool,
                tc.tile_pool(name="roll", bufs=3) as rpool,
                tc.tile_pool(name="psum", bufs=1, space="PSUM") as pp,
            ):
                # ---------------- phase P0: layer-0 projections ----------------
                # full kv0 table (replicated compute), own-range q0 + skip0
                ntile = NTAB // 128
                for rtg in range(ntile // 4):
                    xtg = rpool.tile([128, 512], BF16, tag="xtg")
                    nc.sync.dma_start(out=xtg[:], in_=xT[:, rtg * 512:(rtg + 1) * 512])
                    for sub in range(4):
                        rt = rtg * 4 + sub
                        pk = pp.tile([128, D], F32, tag="pk")
                        pv = pp.tile([128, D], F32, tag="pv")
                        lhsT = xtg[:, sub * 128:(sub + 1) * 128]
                        nc.tensor.matmul(pk[:], lhsT, w0kv_s[:, 0:D], start=True, stop=True)
                        nc.tensor.matmul(pv[:], lhsT, w0kv_s[:, D:2 * D], start=True, stop=True)
                        kvb = rpool.tile([128, 2 * D], BF16, tag="kvb")
                        nc.scalar.activation(kvb[:, 0:D], pk[:], AF.Copy)
                        nc.vector.tensor_copy(out=kvb[:, D:2 * D], in_=pv[:])
                        nc.sync.dma_start(out=kv0_t[rt * 128:(rt + 1) * 128, :], in_=kvb[:])
                for btg in range(NB // 4):
                    xtg = rpool.tile([128, 512], BF16, tag="xtg")
                    nc.sync.dma_start(out=xtg[:], in_=xT_own[:, btg * 512:(btg + 1) * 512])
                    for sub in range(4):
                        bt = btg * 4 + sub
                        lhsT = xtg[:, sub * 128:(sub + 1) * 128]
                        pk = pp.tile([128, D], F32, tag="pk")
                        nc.tensor.matmul(pk[:], lhsT, w0q_s[:], start=True, stop=True)
                        qb = rpool.tile([128, D], BF16, tag="kvb")
                        nc.vector.tensor_tensor(out=qb[:], in0=pk[:], in1=b0q_s[:], op=OP.add)
                        nc.sync.dma_start(out=q0_t[bt * 128:(bt + 1) * 128, :], in_=qb[:])
                        pv = pp.tile([128, D], F32, tag="pv")
                        nc.tensor.matmul(pv[:], lhsT, w0s_s[:], start=True, stop=True)
                        skb = rpool.tile([128, D], F32, tag="skb")
                        nc.scalar.activation(skb[:], pv[:], AF.Copy)
                        nc.sync.dma_start(out=sk0_t[bt * 128:(bt + 1) * 128, :], in_=skb[:])

                # ---------------- edge phase (both layers) ----------------
                def edge_layer(layer):
                    kv_tab = kv0_t if layer == 0 else kv1_t
                    q_tab = q0_t if layer == 0 else q1_t
                    for b in range(NB):
                        ikv = rpool.tile([128, NI // 16], I16, tag="ikv")
                        nc.sync.dma_start(out=ikv[:], in_=kvidx[b:b + 1].rearrange("o p s -> (o p) s"))
                        iq = rpool.tile([128, NI // 16], I16, tag="iq")
                        nc.sync.dma_start(out=iq[:], in_=qidx[b:b + 1].rearrange("o p s -> (o p) s"))
                        dr = rpool.tile([128, T], BF16, tag="dr")
                        nc.sync.dma_start(out=dr[:], in_=dstrel[b:b + 1].rearrange("o p s -> (o p) s"))
                        # dma_gather tops out at 1024 indices per call; split into
                        # sub-gathers of up to 8 tiles (128 idxs each).
                        kvt = pool.tile([128, T, 2 * D], BF16, tag="kvt")
                        qt = pool.tile([128, T, D], BF16, tag="qt")
                        for g0 in range(0, T, 8):
                            gt = min(8, T - g0)
                            ni = gt * 128
                            nc.gpsimd.dma_gather(
                                kvt[:, g0:g0 + gt, :], kv_tab[:],
                                ikv[:, g0 * 8:g0 * 8 + ni // 16], ni, ni,
                                elem_size=2 * D)
                            nc.gpsimd.dma_gather(
                                qt[:, g0:g0 + gt, :], q_tab[:],
                                iq[:, g0 * 8:g0 * 8 + ni // 16], ni, ni,
                                elem_size=D)

                        alpha = pool.tile([128, 4 * T], F32, tag="alpha")
                        for j in range(T):
                            qk = rpool.tile([128, D], BF16, tag="qk")
                            nc.vector.tensor_tensor(
                                out=qk[:], in0=qt[:, j:j + 1, :].rearrange("p o d -> p (o d)"),
                                in1=kvt[:, j:j + 1, 0:D].rearrange("p o d -> p (o d)"),
                                op=OP.mult)
                            nc.vector.tensor_reduce(
                                out=alpha[:, 4 * j:4 * j + 4],
                                in_=qk[:].rearrange("p (h c) -> p h c", c=C),
                                axis=mybir.AxisListType.X, op=OP.add)
                        ae = pool.tile([128, 4 * T], F32, tag="ae")
                        nc.scalar.activation(ae[:], alpha[:], AF.Exp, scale=SCALE)
                        aeb = pool.tile([128, 4 * T], BF16, tag="aeb")
                        nc.vector.tensor_copy(out=aeb[:], in_=ae[:])

                        po = pp.tile([128, D], F32, tag="po")
                        ps = pp.tile([128, 4], F32, tag="ps")
                        for j in range(T):
                            M = rpool.tile([128, 128], BF16, tag="M")
                            nc.vector.tensor_tensor(
                                out=M[:], in0=dr[:, j:j + 1].to_broadcast([128, 128]),
                                in1=iota_s[:], op=OP.is_equal)
                            nc.tensor.matmul(ps[:], M[:], aeb[:, 4 * j:4 * j + 4],
                                             start=(j == 0), stop=(j == T - 1))
                            ct = rpool.tile([128, D], BF16, tag="ct")
                            for h in range(H):
                                nc.vector.tensor_scalar_mul(
                                    ct[:, h * C:(h + 1) * C],
                                    kvt[:, j:j + 1, D + h * C:D + (h + 1) * C].rearrange("p o d -> p (o d)"),
                                    ae[:, 4 * j + h:4 * j + h + 1])
                            nc.tensor.matmul(po[:], M[:], ct[:],
                                             start=(j == 0), stop=(j == T - 1))

                        # block finalize
                        se = rpool.tile([128, 4], F32, tag="se")
                        nc.vector.tensor_scalar_add(se[:], ps[:], 1e-30)
                        iv = rpool.tile([128, 4], F32, tag="iv")
                        nc.vector.reciprocal(out=iv[:], in_=se[:])
                        if layer == 0:
                            hb = rpool.tile([128, D], F32, tag="hb")
                            for h in range(H):
                                nc.vector.tensor_scalar_mul(
                                    hb[:, h * C:(h + 1) * C], po[:, h * C:(h + 1) * C],
                                    iv[:, h:h + 1])
                            skb = rpool.tile([128, D], F32, tag="skb")
                            nc.sync.dma_start(out=skb[:], in_=sk0_t[b * 128:(b + 1) * 128, :])
                            nc.vector.tensor_tensor(out=hb[:], in0=hb[:], in1=skb[:], op=OP.add)
                            nc.vector.tensor_tensor(out=hb[:], in0=hb[:], in1=c0_s[:], op=OP.add)
                            hbb = rpool.tile([128, D], BF16, tag="hbb")
                            nc.scalar.activation(hbb[:], hb[:], AF.Relu)
                            cag = b // AGB
                            hTcs = []
                            for sg in range(4):
                                pt = pp.tile([128, 128], BF16, tag="pa")
                                nc.tensor.transpose(pt[:], hbb[:, sg * 128:(sg + 1) * 128], ident_s[:])
                                hTc = rpool.tile([128, 128], BF16, tag=f"hTc{sg}")
                                nc.scalar.activation(hTc[:], pt[:], AF.Copy)
                                nc.sync.dma_start(
                                    out=hT_own[cag][sg * 128:(sg + 1) * 128,
                                                    (b % AGB) * 128:(b % AGB + 1) * 128],
                                    in_=hTc[:])
                                hTcs.append(hTc)
                            # q1 row-block for this b (uses hT chunks still in SBUF)
                            pq = pp.tile([128, D], F32, tag="pa")
                            for sg in range(4):
                                nc.tensor.matmul(pq[:], hTcs[sg][:], w1q_s[:, sg:sg + 1, :].rearrange("p o d -> p (o d)"),
                                                 start=(sg == 0), stop=(sg == 3))
                            qb = rpool.tile([128, D], BF16, tag="qb1")
                            nc.vector.tensor_tensor(out=qb[:], in0=pq[:], in1=b1q_s[:], op=OP.add)
                            nc.sync.dma_start(out=q1_t[b * 128:(b + 1) * 128, :], in_=qb[:])

                            # allgather + replicated kv1 for completed chunk
                            if (b + 1) % AGB == 0:
                                nc.gpsimd.collective_compute(
                                    "AllGather", OP.bypass, replica_groups=rg,
                                    ins=[hT_own[cag].opt()], outs=[hT_ag[cag].opt()])
                                for r in range(NRANKS):
                                    lhT = pool.tile([128, 4, AGW], BF16, tag="lhT")
                                    nc.sync.dma_start(
                                        out=lhT[:],
                                        in_=hT_ag[cag][r:r + 1].rearrange(
                                            "o (s c) n -> (o c) s n", c=128))
                                    for jt in range(AGB):
                                        pkk = pp.tile([128, D], F32, tag="pkk")
                                        pvv = pp.tile([128, D], F32, tag="pvv")
                                        for sg in range(4):
                                            lhsT = lhT[:, sg:sg + 1, jt * 128:(jt + 1) * 128].rearrange("p o d -> p (o d)")
                                            nc.tensor.matmul(pkk[:], lhsT, w1kv_s[:, sg:sg + 1, 0:D].rearrange("p o d -> p (o d)"),
                                                             start=(sg == 0), stop=(sg == 3))
                                            nc.tensor.matmul(pvv[:], lhsT, w1kv_s[:, sg:sg + 1, D:2 * D].rearrange("p o d -> p (o d)"),
                                                             start=(sg == 0), stop=(sg == 3))
                                        kvb = rpool.tile([128, 2 * D], BF16, tag="kvb1")
                                        nc.scalar.activation(kvb[:, 0:D], pkk[:], AF.Copy)
                                        nc.scalar.activation(kvb[:, D:2 * D], pvv[:], AF.Copy)
                                        row0 = r * PERP + cag * AGW + jt * 128
                                        nc.sync.dma_start(out=kv1_t[row0:row0 + 128, :], in_=kvb[:])
                        else:
                            iv4 = rpool.tile([128, 4], F32, tag="iv4")
                            nc.vector.tensor_scalar_mul(iv4[:], iv[:], 1.0 / H)
                            acc = rpool.tile([128, C], F32, tag="acc")
                            nc.vector.tensor_scalar_mul(acc[:], po[:, 0:C], iv4[:, 0:1])
                            for h in range(1, H):
                                tmp = rpool.tile([128, C], F32, tag="tmp")
                                nc.vector.tensor_scalar_mul(tmp[:], po[:, h * C:(h + 1) * C],
                                                            iv4[:, h:h + 1])
                                nc.vector.tensor_tensor(out=acc[:], in0=acc[:], in1=tmp[:], op=OP.add)
                            lhb = rpool.tile([128, 4, 128], BF16, tag="lhb")
                            nc.sync.dma_start(
                                out=lhb[:],
                                in_=hT_own[b // AGB][:, (b % AGB) * 128:(b % AGB + 1) * 128]
                                .rearrange("(s c) n -> c s n", c=128))
                            psk = pp.tile([128, OUT_CH], F32, tag="pa")
                            for sg in range(4):
                                nc.tensor.matmul(psk[:], lhb[:, sg:sg + 1, :].rearrange("p o d -> p (o d)"),
                                                 w1s_s[:, sg:sg + 1, :].rearrange("p o d -> p (o d)"),
                                                 start=(sg == 0), stop=(sg == 3))
                            ob = rpool.tile([128, OUT_CH], F32, tag="ob")
                            nc.vector.tensor_tensor(out=ob[:], in0=acc[:], in1=psk[:], op=OP.add)
                            nc.vector.tensor_tensor(out=ob[:], in0=ob[:], in1=c1_s[:], op=OP.add)
                            nc.sync.dma_start(out=out_t[b * 128:(b + 1) * 128, :], in_=ob[:])

                edge_layer(0)
                edge_layer(1)

    nc.compile()
    return nc


# ----------------------------------------------------------------------------
# Host-side preparation
# ----------------------------------------------------------------------------

def host_prep(cfg, x, edge_index,
              Wq0, bq0, Wk0, bk0, Wv0, bv0, Ws0, bs0,
              Wq1, bq1, Wk1, bk1, Wv1, bv1, Ws1, bs1):
    g = derive(cfg)
    N, NRANKS, NB, T, NI, PER, PERP, NTAB = (
        g["N"], g["RANKS"], g["NB"], g["T"], g["NI"], g["PER"], g["PERP"], g["NTAB"])

    x = np.asarray(x, np.float32)
    src = np.asarray(edge_index[0], np.int64)
    dst = np.asarray(edge_index[1], np.int64)

    # n -> padded table id n'
    nprime = (src // PER) * PERP + (src % PER)

    xT = np.zeros((D_IN, NTAB), BF)
    for r in range(NRANKS):
        xT[:, r * PERP:r * PERP + PER] = x[r * PER:(r + 1) * PER].T

    common = dict(
        xT=xT,
        w0kv=np.concatenate([Wk0, Wv0], 1).astype(BF),
        w0q=np.asarray(Wq0, np.float32).astype(BF),
        w0s=np.asarray(Ws0, np.float32).astype(BF),
        w1kv=np.ascontiguousarray(
            np.concatenate([Wk1, Wv1], 1).astype(BF).reshape(4, 128, 2 * D).transpose(1, 0, 2)),
        w1q=np.ascontiguousarray(np.asarray(Wq1, np.float32).astype(BF).reshape(4, 128, D).transpose(1, 0, 2)),
        w1s=np.ascontiguousarray(np.asarray(Ws1, np.float32).astype(BF).reshape(4, 128, OUT_CH).transpose(1, 0, 2)),
        b0q=np.tile(np.asarray(bq0, np.float32)[None], (128, 1)),
        c0=np.tile((np.asarray(bs0) + np.asarray(bv0)).astype(np.float32)[None], (128, 1)),
        b1q=np.tile(np.asarray(bq1, np.float32)[None], (128, 1)),
        c1=np.tile((np.asarray(bs1) + np.asarray(bv1, np.float32).reshape(H, OUT_CH).mean(0))[None],
                   (128, 1)).astype(np.float32),
        iota=np.tile(np.arange(128).astype(BF)[None], (128, 1)),
    )

    def wrap_idx(arr):  # [NB, NI] int -> [NB, 128, NI//16] int16
        a = arr.reshape(NB, NI // 16, 16).transpose(0, 2, 1)  # [NB, 16, NI//16]
        return np.ascontiguousarray(np.tile(a, (1, 8, 1)).astype(np.int16))

    in_maps = []
    for r in range(NRANKS):
        lo, hi = r * PER, (r + 1) * PER
        m = (dst >= lo) & (dst < hi)
        es, ed = src[m], dst[m] - lo
        blk = ed // 128
        order = np.argsort(blk, kind="stable")
        es, ed, blk = es[order], ed[order], blk[order]
        cnt = np.bincount(blk, minlength=NB)
        assert cnt.max() <= NI, f"block overflow: {cnt.max()} > {NI}"
        kvi = np.zeros((NB, NI), np.int64)
        qi = np.zeros((NB, NI), np.int64)
        drl = np.full((NB, NI), -1.0, np.float32)
        pos = 0
        for b in range(NB):
            nb = cnt[b]
            sl = slice(pos, pos + nb)
            kvi[b, :nb] = nprime[m][order][sl]
            qi[b, :nb] = ed[sl]
            drl[b, :nb] = (ed[sl] % 128).astype(np.float32)
            pos += nb
        im = dict(common)
        im["xT_own"] = np.ascontiguousarray(xT[:, r * PERP:(r + 1) * PERP])
        im["kvidx"] = wrap_idx(kvi)
        im["qidx"] = wrap_idx(qi)
        im["dstrel"] = np.ascontiguousarray(
            drl.reshape(NB, T, 128).transpose(0, 2, 1).astype(BF))
        in_maps.append(im)
    return in_maps


# ----------------------------------------------------------------------------
# Entry point
# ----------------------------------------------------------------------------

_CACHE = {}


def _get_program():
    if "nc" not in _CACHE:
        _CACHE["nc"] = build_program(FULL_CFG)
    return _CACHE["nc"]


def run_on_hw(nc, in_maps, cfg, trace=False):
    from concourse import bass_utils
    g = derive(cfg)
    res = bass_utils.run_bass_kernel_spmd(
        nc, in_maps, core_ids=list(range(g["RANKS"])), trace=trace)
    outs = [res.results[r]["out"][:g["PER"]] for r in range(g["RANKS"])]
    return np.concatenate(outs, 0).astype(np.float32), res


def kernel(x, edge_index,
           Wq0, bq0, Wk0, bk0, Wv0, bv0, Ws0, bs0,
           Wq1, bq1, Wk1, bk1, Wv1, bv1, Ws1, bs1):
    nc = _get_program()
    in_maps = host_prep(FULL_CFG, x, edge_index,
                        Wq0, bq0, Wk0, bk0, Wv0, bv0, Ws0, bs0,
                        Wq1, bq1, Wk1, bk1, Wv1, bv1, Ws1, bs1)
    out, _ = run_on_hw(nc, in_maps, FULL_CFG)
    return out


# revision 12
# speedup vs baseline: 1.6373x; 1.6373x over previous
"""nn_GT_7327214207519 — 2-layer TransformerConv GNN (heads=4) on 8 trn2 NeuronCores.

Strategy (edge/dst-parallel):
  * Nodes are split into 8 contiguous ranges (2500/core). Each core owns the
    destination-side softmax + aggregation for its node range, so no
    cross-core reduction is needed for the attention itself.
  * Per core, dst nodes are grouped into NB blocks of 128. Each block's edges
    (padded to T*128 slots) are fetched with dma_gathers of interleaved
    [K|V] rows; logits, exp and the weighted message sums are computed per
    128-edge tile, with a dst-match matrix M (via is_equal against an iota
    row) so TensorE matmuls accumulate per-destination sums in PSUM.
  * Softmax max-subtraction is skipped (logits are O(1) for this model), and
    biases are folded: bq into the q table, bk cancels in softmax, bv/bs into
    per-block constant rows.
  * Layer-1 k/v tables are computed replicated on every core from an
    AllGather of the transposed hidden h (bf16, chunked, overlapped with the
    layer-0 edge phase).
  * Host<->device traffic is minimized for the slow axon tunnel: each core
    receives ONE uint8 blob (~2.5MB) holding its x-rows, its 1/8 weight
    shard, and its edge indices; x and the weights are AllGathered on-device
    instead of being replicated over the tunnel. Output is bf16.
"""

import math
import numpy as np
import ml_dtypes

BF = ml_dtypes.bfloat16

# Problem constants (fixed by the task; kernel.py must be self-contained).
N_NODES, N_EDGES, D_IN, HID, OUT_CH, H = 20000, 320000, 128, 128, 128, 4
C = 128            # per-head channels, both layers
D = H * C          # 512
RANKS = 8

FULL_CFG = dict(N=N_NODES, RANKS=RANKS, NB=20, T=17, AGC=4)

# weights blob: w0kv [128,2D] | w0q [128,D] | w0s [128,D] |
#               w1kv [128,4,2D] | w1q [128,4,D] | w1s [128,4,OUT_CH]   (bf16)
W_SIZES = [128 * 2 * D, 128 * D, 128 * D, 128 * 4 * 2 * D, 128 * 4 * D,
           128 * 4 * OUT_CH]
WTOT = sum(W_SIZES) * 2            # bytes
W_OFFS = np.cumsum([0] + W_SIZES)[:-1] * 2


def derive(cfg):
    g = dict(cfg)
    g["PER"] = g["N"] // g["RANKS"]          # real nodes per rank
    g["PERP"] = g["NB"] * 128                # padded nodes per rank
    assert g["PERP"] >= g["PER"]
    g["NI"] = g["T"] * 128                   # edge slots per block
    g["NTAB"] = g["RANKS"] * g["PERP"]       # padded table rows
    assert g["NB"] % g["AGC"] == 0
    g["AGB"] = g["NB"] // g["AGC"]           # blocks per allgather chunk
    g["AGW"] = g["AGB"] * 128                # nodes per allgather chunk
    assert g["NTAB"] < 32768                 # int16 gather indices
    assert WTOT % g["RANKS"] == 0
    g["WSH"] = WTOT // g["RANKS"]            # weight shard bytes per rank
    # input blob layout (bytes)
    sizes = dict(
        x_own=g["PERP"] * D_IN * 2,
        wsh=g["WSH"],
        kvidx=g["NB"] * 128 * (g["NI"] // 16) * 2,
        qidx=g["NB"] * 128 * (g["NI"] // 16) * 2,
        dstrel=g["NB"] * 128 * g["T"] * 2,
        bias=(3 * D + OUT_CH) * 4,
        iota=128 * 128 * 2,
    )
    offs, off = {}, 0
    for k, s in sizes.items():
        offs[k] = off
        off += s
    g["BLOB_OFFS"], g["BLOB_BYTES"] = offs, off
    return g


# ----------------------------------------------------------------------------
# Program builder
# ----------------------------------------------------------------------------

def build_program(cfg):
    import concourse.bass as bass
    import concourse.mybir as mybir
    import concourse.tile as tile
    from concourse import bacc
    from concourse.masks import make_identity

    g = derive(cfg)
    NB, T, NI, NTAB, PERP, AGC, AGB, AGW, WSH = (
        g["NB"], g["T"], g["NI"], g["NTAB"], g["PERP"], g["AGC"], g["AGB"],
        g["AGW"], g["WSH"])
    NRANKS = g["RANKS"]
    OFFS = g["BLOB_OFFS"]
    F32, BF16, I16, U8 = (mybir.dt.float32, mybir.dt.bfloat16, mybir.dt.int16,
                          mybir.dt.uint8)
    AF = mybir.ActivationFunctionType
    OP = mybir.AluOpType
    SCALE = 1.0 / math.sqrt(C)

    nc = bacc.Bacc("TRN2", target_bir_lowering=False, debug=False,
                   num_devices=NRANKS)

    blob = nc.dram_tensor("blob", [g["BLOB_BYTES"]], U8, kind="ExternalInput").ap()
    out_t = nc.dram_tensor("out", [PERP, OUT_CH], BF16, kind="ExternalOutput").ap()

    def bv(key, nbytes=None, off_extra=0):
        o = OFFS[key] + off_extra
        n = nbytes
        return blob[o:o + n]

    rg = [list(range(NRANKS))]

    with tile.TileContext(nc) as tc:
        with (
            tc.tile_pool(name="dram", bufs=1, space="DRAM") as dpool,
            tc.tile_pool(name="const", bufs=1) as cpool,
        ):
            kv0_t = dpool.tile([NTAB, 2 * D], BF16, tag="kv0")
            q0_t = dpool.tile([PERP, D], BF16, tag="q0")
            sk0_t = dpool.tile([PERP, D], F32, tag="sk0")
            kv1_t = dpool.tile([NTAB, 2 * D], BF16, tag="kv1")
            q1_t = dpool.tile([PERP, D], BF16, tag="q1")
            hT_own = [dpool.tile([D, AGW], BF16, tag=f"hTo{c}", name=f"hTo{c}")
                      for c in range(AGC)]
            hT_ag = [dpool.tile([NRANKS, D, AGW], BF16, tag=f"hTa{c}", name=f"hTa{c}",
                                addr_space="Shared")
                     for c in range(AGC)]
            # bounce + allgather buffers for x rows and the weight shard
            xb = dpool.tile([PERP, D_IN], BF16, tag="xb")
            xfull = dpool.tile([NTAB, D_IN], BF16, tag="xfull", addr_space="Shared")
            wb = dpool.tile([WSH], U8, tag="wb")
            wfull = dpool.tile([NRANKS * WSH], U8, tag="wfull", addr_space="Shared")

            nc.sync.dma_start(out=xb[:], in_=bv("x_own", PERP * D_IN * 2)
                              .bitcast(BF16).rearrange("(n d) -> n d", d=D_IN))
            nc.gpsimd.collective_compute(
                "AllGather", OP.bypass, replica_groups=rg,
                ins=[xb.opt()], outs=[xfull.opt()])
            nc.sync.dma_start(out=wb[:], in_=bv("wsh", WSH))
            nc.gpsimd.collective_compute(
                "AllGather", OP.bypass, replica_groups=rg,
                ins=[wb.opt()], outs=[wfull.opt()])

            def wview(i, shape_str, **kw):
                v = wfull[int(W_OFFS[i]):int(W_OFFS[i]) + W_SIZES[i] * 2]
                return v.bitcast(BF16).rearrange(shape_str, **kw)

            w0kv_s = cpool.tile([128, 2 * D], BF16, tag="w0kv")
            nc.sync.dma_start(out=w0kv_s[:], in_=wview(0, "(p d) -> p d", p=128))
            w0q_s = cpool.tile([128, D], BF16, tag="w0q")
            nc.sync.dma_start(out=w0q_s[:], in_=wview(1, "(p d) -> p d", p=128))
            w0s_s = cpool.tile([128, D], BF16, tag="w0s")
            nc.sync.dma_start(out=w0s_s[:], in_=wview(2, "(p d) -> p d", p=128))
            w1kv_s = cpool.tile([128, 4, 2 * D], BF16, tag="w1kv")
            nc.sync.dma_start(out=w1kv_s[:], in_=wview(3, "(p s d) -> p s d", p=128, s=4))
            w1q_s = cpool.tile([128, 4, D], BF16, tag="w1q")
            nc.sync.dma_start(out=w1q_s[:], in_=wview(4, "(p s d) -> p s d", p=128, s=4))
            w1s_s = cpool.tile([128, 4, OUT_CH], BF16, tag="w1s")
            nc.sync.dma_start(out=w1s_s[:], in_=wview(5, "(p s d) -> p s d", p=128, s=4))

            iota_s = cpool.tile([128, 128], BF16, tag="iota")
            nc.sync.dma_start(out=iota_s[:],
                              in_=bv("iota", 128 * 128 * 2).bitcast(BF16)
                              .rearrange("(p d) -> p d", p=128))
            ident_s = cpool.tile([128, 128], BF16, tag="ident")
            make_identity(nc, ident_s[:])

            # broadcast bias rows [1, D] -> [128, D] via ones-matmul
            ones_s = cpool.tile([1, 128], F32, tag="ones")
            nc.vector.memset(ones_s[:], 1.0)
            brow_s = cpool.tile([1, 3 * D + OUT_CH], F32, tag="brow")
            nc.sync.dma_start(out=brow_s[:],
                              in_=bv("bias", (3 * D + OUT_CH) * 4).bitcast(F32)
                              .rearrange("(o d) -> o d", o=1))
            b0q_s = cpool.tile([128, D], F32, tag="b0q")
            c0_s = cpool.tile([128, D], F32, tag="c0")
            b1q_s = cpool.tile([128, D], F32, tag="b1q")
            c1_s = cpool.tile([128, OUT_CH], F32, tag="c1")

            with (
                tc.tile_pool(name="work", bufs=2) as pool,
                tc.tile_pool(name="roll", bufs=3) as rpool,
                tc.tile_pool(name="psum", bufs=1, space="PSUM") as pp,
            ):
                for bi, (btile, w) in enumerate(
                        [(b0q_s, D), (c0_s, D), (b1q_s, D), (c1_s, OUT_CH)]):
                    pb = pp.tile([128, D], F32, tag="pa")
                    nc.tensor.matmul(pb[:, :w], ones_s[:],
                                     brow_s[:, bi * D:bi * D + w], start=True, stop=True)
                    nc.scalar.activation(btile[:], pb[:, :w], AF.Copy)

                # ---------------- phase P0: layer-0 projections ----------------
                ntile = NTAB // 128
                for rtg in range(ntile // 4):
                    xtg = rpool.tile([128, 512], BF16, tag="xtg")
                    nc.sync.dma_start(out=xtg[:],
                                      in_=xfull[rtg * 512:(rtg + 1) * 512, :],
                                      transpose=True)
                    for sub in range(4):
                        rt = rtg * 4 + sub
                        pk = pp.tile([128, D], F32, tag="pk")
                        pv = pp.tile([128, D], F32, tag="pv")
                        lhsT = xtg[:, sub * 128:(sub + 1) * 128]
                        nc.tensor.matmul(pk[:], lhsT, w0kv_s[:, 0:D], start=True, stop=True)
                        nc.tensor.matmul(pv[:], lhsT, w0kv_s[:, D:2 * D], start=True, stop=True)
                        kvb = rpool.tile([128, 2 * D], BF16, tag="kvb")
                        nc.scalar.activation(kvb[:, 0:D], pk[:], AF.Copy)
                        nc.vector.tensor_copy(out=kvb[:, D:2 * D], in_=pv[:])
                        nc.sync.dma_start(out=kv0_t[rt * 128:(rt + 1) * 128, :], in_=kvb[:])
                x_own_v = bv("x_own", PERP * D_IN * 2).bitcast(BF16).rearrange(
                    "(n d) -> n d", d=D_IN)
                for btg in range(NB // 4):
                    xtg = rpool.tile([128, 512], BF16, tag="xtg")
                    nc.sync.dma_start(out=xtg[:],
                                      in_=x_own_v[btg * 512:(btg + 1) * 512, :],
                                      transpose=True)
                    for sub in range(4):
                        bt = btg * 4 + sub
                        lhsT = xtg[:, sub * 128:(sub + 1) * 128]
                        pk = pp.tile([128, D], F32, tag="pk")
                        nc.tensor.matmul(pk[:], lhsT, w0q_s[:], start=True, stop=True)
                        qb = rpool.tile([128, D], BF16, tag="kvb")
                        nc.vector.tensor_tensor(out=qb[:], in0=pk[:], in1=b0q_s[:], op=OP.add)
                        nc.sync.dma_start(out=q0_t[bt * 128:(bt + 1) * 128, :], in_=qb[:])
                        pv = pp.tile([128, D], F32, tag="pv")
                        nc.tensor.matmul(pv[:], lhsT, w0s_s[:], start=True, stop=True)
                        skb = rpool.tile([128, D], F32, tag="skb")
                        nc.scalar.activation(skb[:], pv[:], AF.Copy)
                        nc.sync.dma_start(out=sk0_t[bt * 128:(bt + 1) * 128, :], in_=skb[:])

                kvidx_v = bv("kvidx", NB * 128 * (NI // 16) * 2).bitcast(I16).rearrange(
                    "(b p s) -> b p s", b=NB, p=128)
                qidx_v = bv("qidx", NB * 128 * (NI // 16) * 2).bitcast(I16).rearrange(
                    "(b p s) -> b p s", b=NB, p=128)
                dstrel_v = bv("dstrel", NB * 128 * T * 2).bitcast(BF16).rearrange(
                    "(b p s) -> b p s", b=NB, p=128)

                # ---------------- edge phase (both layers) ----------------
                def edge_layer(layer):
                    kv_tab = kv0_t if layer == 0 else kv1_t
                    q_tab = q0_t if layer == 0 else q1_t
                    for b in range(NB):
                        ikv = rpool.tile([128, NI // 16], I16, tag="ikv")
                        nc.sync.dma_start(out=ikv[:], in_=kvidx_v[b:b + 1].rearrange("o p s -> (o p) s"))
                        iq = rpool.tile([128, NI // 16], I16, tag="iq")
                        nc.sync.dma_start(out=iq[:], in_=qidx_v[b:b + 1].rearrange("o p s -> (o p) s"))
                        dr = rpool.tile([128, T], BF16, tag="dr")
                        nc.sync.dma_start(out=dr[:], in_=dstrel_v[b:b + 1].rearrange("o p s -> (o p) s"))
                        # dma_gather tops out at 1024 indices per call
                        kvt = pool.tile([128, T, 2 * D], BF16, tag="kvt")
                        qt = pool.tile([128, T, D], BF16, tag="qt")
                        for g0 in range(0, T, 8):
                            gt = min(8, T - g0)
                            ni = gt * 128
                            nc.gpsimd.dma_gather(
                                kvt[:, g0:g0 + gt, :], kv_tab[:],
                                ikv[:, g0 * 8:g0 * 8 + ni // 16], ni, ni,
                                elem_size=2 * D)
                            nc.gpsimd.dma_gather(
                                qt[:, g0:g0 + gt, :], q_tab[:],
                                iq[:, g0 * 8:g0 * 8 + ni // 16], ni, ni,
                                elem_size=D)

                        alpha = pool.tile([128, 4 * T], F32, tag="alpha")
                        for j in range(T):
                            qk = rpool.tile([128, D], BF16, tag="qk")
                            nc.vector.tensor_tensor(
                                out=qk[:], in0=qt[:, j:j + 1, :].rearrange("p o d -> p (o d)"),
                                in1=kvt[:, j:j + 1, 0:D].rearrange("p o d -> p (o d)"),
                                op=OP.mult)
                            nc.vector.tensor_reduce(
                                out=alpha[:, 4 * j:4 * j + 4],
                                in_=qk[:].rearrange("p (h c) -> p h c", c=C),
                                axis=mybir.AxisListType.X, op=OP.add)
                        ae = pool.tile([128, 4 * T], F32, tag="ae")
                        nc.scalar.activation(ae[:], alpha[:], AF.Exp, scale=SCALE)
                        aeb = pool.tile([128, 4 * T], BF16, tag="aeb")
                        nc.vector.tensor_copy(out=aeb[:], in_=ae[:])

                        po = pp.tile([128, D], F32, tag="po")
                        ps = pp.tile([128, 4], F32, tag="ps")
                        for j in range(T):
                            M = rpool.tile([128, 128], BF16, tag="M")
                            nc.vector.tensor_tensor(
                                out=M[:], in0=dr[:, j:j + 1].to_broadcast([128, 128]),
                                in1=iota_s[:], op=OP.is_equal)
                            nc.tensor.matmul(ps[:], M[:], aeb[:, 4 * j:4 * j + 4],
                                             start=(j == 0), stop=(j == T - 1))
                            ct = rpool.tile([128, D], BF16, tag="ct")
                            for h in range(H):
                                nc.vector.tensor_scalar_mul(
                                    ct[:, h * C:(h + 1) * C],
                                    kvt[:, j:j + 1, D + h * C:D + (h + 1) * C].rearrange("p o d -> p (o d)"),
                                    ae[:, 4 * j + h:4 * j + h + 1])
                            nc.tensor.matmul(po[:], M[:], ct[:],
                                             start=(j == 0), stop=(j == T - 1))

                        # block finalize
                        se = rpool.tile([128, 4], F32, tag="se")
                        nc.vector.tensor_scalar_add(se[:], ps[:], 1e-30)
                        iv = rpool.tile([128, 4], F32, tag="iv")
                        nc.vector.reciprocal(out=iv[:], in_=se[:])
                        if layer == 0:
                            hb = rpool.tile([128, D], F32, tag="hb")
                            for h in range(H):
                                nc.vector.tensor_scalar_mul(
                                    hb[:, h * C:(h + 1) * C], po[:, h * C:(h + 1) * C],
                                    iv[:, h:h + 1])
                            skb = rpool.tile([128, D], F32, tag="skb")
                            nc.sync.dma_start(out=skb[:], in_=sk0_t[b * 128:(b + 1) * 128, :])
                            nc.vector.tensor_tensor(out=hb[:], in0=hb[:], in1=skb[:], op=OP.add)
                            nc.vector.tensor_tensor(out=hb[:], in0=hb[:], in1=c0_s[:], op=OP.add)
                            hbb = rpool.tile([128, D], BF16, tag="hbb")
                            nc.scalar.activation(hbb[:], hb[:], AF.Relu)
                            cag = b // AGB
                            hTcs = []
                            for sg in range(4):
                                pt = pp.tile([128, 128], BF16, tag="pa")
                                nc.tensor.transpose(pt[:], hbb[:, sg * 128:(sg + 1) * 128], ident_s[:])
                                hTc = rpool.tile([128, 128], BF16, tag=f"hTc{sg}")
                                nc.scalar.activation(hTc[:], pt[:], AF.Copy)
                                nc.sync.dma_start(
                                    out=hT_own[cag][sg * 128:(sg + 1) * 128,
                                                    (b % AGB) * 128:(b % AGB + 1) * 128],
                                    in_=hTc[:])
                                hTcs.append(hTc)
                            # q1 row-block for this b (uses hT chunks still in SBUF)
                            pq = pp.tile([128, D], F32, tag="pa")
                            for sg in range(4):
                                nc.tensor.matmul(pq[:], hTcs[sg][:], w1q_s[:, sg:sg + 1, :].rearrange("p o d -> p (o d)"),
                                                 start=(sg == 0), stop=(sg == 3))
                            qb = rpool.tile([128, D], BF16, tag="qb1")
                            nc.vector.tensor_tensor(out=qb[:], in0=pq[:], in1=b1q_s[:], op=OP.add)
                            nc.sync.dma_start(out=q1_t[b * 128:(b + 1) * 128, :], in_=qb[:])

                            # allgather + replicated kv1 for completed chunk
                            if (b + 1) % AGB == 0:
                                nc.gpsimd.collective_compute(
                                    "AllGather", OP.bypass, replica_groups=rg,
                                    ins=[hT_own[cag].opt()], outs=[hT_ag[cag].opt()])
                                for r in range(NRANKS):
                                    lhT = pool.tile([128, 4, AGW], BF16, tag="lhT")
                                    nc.sync.dma_start(
                                        out=lhT[:],
                                        in_=hT_ag[cag][r:r + 1].rearrange(
                                            "o (s c) n -> (o c) s n", c=128))
                                    for jt in range(AGB):
                                        pkk = pp.tile([128, D], F32, tag="pkk")
                                        pvv = pp.tile([128, D], F32, tag="pvv")
                                        for sg in range(4):
                                            lhsT = lhT[:, sg:sg + 1, jt * 128:(jt + 1) * 128].rearrange("p o d -> p (o d)")
                                            nc.tensor.matmul(pkk[:], lhsT, w1kv_s[:, sg:sg + 1, 0:D].rearrange("p o d -> p (o d)"),
                                                             start=(sg == 0), stop=(sg == 3))
                                            nc.tensor.matmul(pvv[:], lhsT, w1kv_s[:, sg:sg + 1, D:2 * D].rearrange("p o d -> p (o d)"),
                                                             start=(sg == 0), stop=(sg == 3))
                                        kvb = rpool.tile([128, 2 * D], BF16, tag="kvb1")
                                        nc.scalar.activation(kvb[:, 0:D], pkk[:], AF.Copy)
                                        nc.scalar.activation(kvb[:, D:2 * D], pvv[:], AF.Copy)
                                        row0 = r * PERP + cag * AGW + jt * 128
                                        nc.sync.dma_start(out=kv1_t[row0:row0 + 128, :], in_=kvb[:])
                        else:
                            iv4 = rpool.tile([128, 4], F32, tag="iv4")
                            nc.vector.tensor_scalar_mul(iv4[:], iv[:], 1.0 / H)
                            acc = rpool.tile([128, C], F32, tag="acc")
                            nc.vector.tensor_scalar_mul(acc[:], po[:, 0:C], iv4[:, 0:1])
                            for h in range(1, H):
                                tmp = rpool.tile([128, C], F32, tag="tmp")
                                nc.vector.tensor_scalar_mul(tmp[:], po[:, h * C:(h + 1) * C],
                                                            iv4[:, h:h + 1])
                                nc.vector.tensor_tensor(out=acc[:], in0=acc[:], in1=tmp[:], op=OP.add)
                            lhb = rpool.tile([128, 4, 128], BF16, tag="lhb")
                            nc.sync.dma_start(
                                out=lhb[:],
                                in_=hT_own[b // AGB][:, (b % AGB) * 128:(b % AGB + 1) * 128]
                                .rearrange("(s c) n -> c s n", c=128))
                            psk = pp.tile([128, OUT_CH], F32, tag="pa")
                            for sg in range(4):
                                nc.tensor.matmul(psk[:], lhb[:, sg:sg + 1, :].rearrange("p o d -> p (o d)"),
                                                 w1s_s[:, sg:sg + 1, :].rearrange("p o d -> p (o d)"),
                                                 start=(sg == 0), stop=(sg == 3))
                            ob = rpool.tile([128, OUT_CH], BF16, tag="ob")
                            nc.vector.tensor_tensor(out=ob[:], in0=acc[:], in1=psk[:], op=OP.add)
                            nc.vector.tensor_tensor(out=ob[:], in0=ob[:], in1=c1_s[:], op=OP.add)
                            nc.sync.dma_start(out=out_t[b * 128:(b + 1) * 128, :], in_=ob[:])

                edge_layer(0)
                edge_layer(1)

    nc.compile()
    return nc


# ----------------------------------------------------------------------------
# Host-side preparation
# ----------------------------------------------------------------------------

def host_prep(cfg, x, edge_index,
              Wq0, bq0, Wk0, bk0, Wv0, bv0, Ws0, bs0,
              Wq1, bq1, Wk1, bk1, Wv1, bv1, Ws1, bs1):
    g = derive(cfg)
    N, NRANKS, NB, T, NI, PER, PERP = (
        g["N"], g["RANKS"], g["NB"], g["T"], g["NI"], g["PER"], g["PERP"])
    OFFS, TOT = g["BLOB_OFFS"], g["BLOB_BYTES"]

    x = np.asarray(x, np.float32)
    src = np.asarray(edge_index[0], np.int64)
    dst = np.asarray(edge_index[1], np.int64)
    nprime = (src // PER) * PERP + (src % PER)   # node id in padded tables

    w0kv = np.concatenate([np.asarray(Wk0, np.float32).astype(BF),
                           np.asarray(Wv0, np.float32).astype(BF)], 1)
    wblob = np.concatenate([
        w0kv.reshape(-1),
        np.asarray(Wq0, np.float32).astype(BF).reshape(-1),
        np.asarray(Ws0, np.float32).astype(BF).reshape(-1),
        np.ascontiguousarray(
            np.concatenate([Wk1, Wv1], 1).astype(BF).reshape(4, 128, 2 * D)
            .transpose(1, 0, 2)).reshape(-1),
        np.ascontiguousarray(np.asarray(Wq1, np.float32).astype(BF)
                             .reshape(4, 128, D).transpose(1, 0, 2)).reshape(-1),
        np.ascontiguousarray(np.asarray(Ws1, np.float32).astype(BF)
                             .reshape(4, 128, OUT_CH).transpose(1, 0, 2)).reshape(-1),
    ]).view(np.uint8)
    assert wblob.nbytes == WTOT, (wblob.nbytes, WTOT)

    bias = np.concatenate([
        np.asarray(bq0, np.float32),
        (np.asarray(bs0) + np.asarray(bv0)).astype(np.float32),
        np.asarray(bq1, np.float32),
        (np.asarray(bs1) + np.asarray(bv1, np.float32).reshape(H, OUT_CH).mean(0))
        .astype(np.float32),
    ]).view(np.uint8)
    iota = np.tile(np.arange(128).astype(BF)[None], (128, 1)).view(np.uint8).reshape(-1)

    def wrap_idx(arr):  # [NB, NI] int -> [NB, 128, NI//16] int16 bytes
        a = arr.reshape(NB, NI // 16, 16).transpose(0, 2, 1)
        return np.ascontiguousarray(np.tile(a, (1, 8, 1)).astype(np.int16))

    in_maps = []
    for r in range(NRANKS):
        lo, hi = r * PER, (r + 1) * PER
        m = (dst >= lo) & (dst < hi)
        es, ed = src[m], dst[m] - lo
        npr = nprime[m]
        blk = ed // 128
        order = np.argsort(blk, kind="stable")
        es, ed, npr, blk = es[order], ed[order], npr[order], blk[order]
        cnt = np.bincount(blk, minlength=NB)
        assert cnt.max() <= NI, f"block overflow: {cnt.max()} > {NI}"
        kvi = np.zeros((NB, NI), np.int64)
        qi = np.zeros((NB, NI), np.int64)
        drl = np.full((NB, NI), -1.0, np.float32)
        pos = 0
        for b in range(NB):
            nb = cnt[b]
            sl = slice(pos, pos + nb)
            kvi[b, :nb] = npr[sl]
            qi[b, :nb] = ed[sl]
            drl[b, :nb] = (ed[sl] % 128).astype(np.float32)
            pos += nb
        xo = np.zeros((PERP, D_IN), BF)
        xo[:PER] = x[lo:hi]
        blob = np.empty(TOT, np.uint8)
        blob[OFFS["x_own"]:OFFS["x_own"] + xo.nbytes] = xo.view(np.uint8).reshape(-1)
        blob[OFFS["wsh"]:OFFS["wsh"] + g["WSH"]] = wblob[r * g["WSH"]:(r + 1) * g["WSH"]]
        kb = wrap_idx(kvi).view(np.uint8).reshape(-1)
        blob[OFFS["kvidx"]:OFFS["kvidx"] + kb.nbytes] = kb
        qb = wrap_idx(qi).view(np.uint8).reshape(-1)
        blob[OFFS["qidx"]:OFFS["qidx"] + qb.nbytes] = qb
        db = np.ascontiguousarray(
            drl.reshape(NB, T, 128).transpose(0, 2, 1).astype(BF)).view(np.uint8).reshape(-1)
        blob[OFFS["dstrel"]:OFFS["dstrel"] + db.nbytes] = db
        blob[OFFS["bias"]:OFFS["bias"] + bias.nbytes] = bias
        blob[OFFS["iota"]:OFFS["iota"] + iota.nbytes] = iota
        in_maps.append(dict(blob=blob))
    return in_maps


# ----------------------------------------------------------------------------
# Entry point
# ----------------------------------------------------------------------------

_CACHE = {}


def _get_program():
    if "nc" not in _CACHE:
        _CACHE["nc"] = build_program(FULL_CFG)
    return _CACHE["nc"]


def run_on_hw(nc, in_maps, cfg, trace=False):
    from concourse import bass_utils
    g = derive(cfg)
    res = bass_utils.run_bass_kernel_spmd(
        nc, in_maps, core_ids=list(range(g["RANKS"])), trace=trace)
    outs = [np.asarray(res.results[r]["out"][:g["PER"]], np.float32)
            for r in range(g["RANKS"])]
    return np.concatenate(outs, 0), res


def kernel(x, edge_index,
           Wq0, bq0, Wk0, bk0, Wv0, bv0, Ws0, bs0,
           Wq1, bq1, Wk1, bk1, Wv1, bv1, Ws1, bs1):
    nc = _get_program()
    in_maps = host_prep(FULL_CFG, x, edge_index,
                        Wq0, bq0, Wk0, bk0, Wv0, bv0, Ws0, bs0,
                        Wq1, bq1, Wk1, bk1, Wv1, bv1, Ws1, bs1)
    out, _ = run_on_hw(nc, in_maps, FULL_CFG)
    return out


# revision 23
# speedup vs baseline: 1.7695x; 1.0808x over previous
"""nn_GT_7327214207519 — 2-layer TransformerConv GNN (heads=4) on 8 trn2 NeuronCores.

Design notes (this runtime executes roughly one instruction per ~70us, so the
program is shaped to minimize INSTRUCTION COUNT above all):
  * Nodes are split into 8 contiguous ranges (2500/core); each core owns the
    destination-side softmax + aggregation for its range (no cross-core
    reduction for attention).
  * Edges are processed in flat chunks of 1024: one dma_gather for [K|V] rows,
    one for q rows, a handful of batched strided/broadcast DVE ops for the
    logits/exp/weighted messages, then ONE dma_scatter_add that accumulates
    [alpha*v | alpha] rows per destination in an HBM table. A dense pass then
    normalizes per destination.
  * Softmax max-subtraction is skipped (logits are O(1) here); biases are
    folded (bq into q table, bk cancels in softmax, bv/bs into dense adds).
  * k/v tables are computed sharded (own rows only) and AllGathered on-device;
    x and the weights also arrive sharded and are AllGathered — the
    host<->device tunnel is slow, so each core receives ONE ~2.3MB blob.
"""

import math
import numpy as np
import ml_dtypes

BF = ml_dtypes.bfloat16

# Problem constants (fixed by the task; kernel.py must be self-contained).
N_NODES, N_EDGES, D_IN, HID, OUT_CH, H = 20000, 320000, 128, 128, 128, 4
C = 128            # per-head channels, both layers
D = H * C          # 512
RANKS = 8
TW = 576           # scatter-table row: [alpha*v (512) | alpha (4) | pad] f32; 2304B % 256 == 0

FULL_CFG = dict(N=N_NODES, RANKS=RANKS, NB=20, T=17)

# weights blob: w0kv [128,2D] | w0q [128,D] | w0s [128,D] |
#               w1kv [128,4,2D] | w1q [128,4,D] | w1s [128,4,OUT_CH]   (bf16)
W_SIZES = [128 * 2 * D, 128 * D, 128 * D, 128 * 4 * 2 * D, 128 * 4 * D,
           128 * 4 * OUT_CH]
WTOT = sum(W_SIZES) * 2            # bytes
W_OFFS = np.cumsum([0] + W_SIZES)[:-1] * 2


def derive(cfg):
    g = dict(cfg)
    g["PER"] = g["N"] // g["RANKS"]          # real nodes per rank
    g["PERP"] = g["NB"] * 128                # padded nodes per rank
    assert g["PERP"] >= g["PER"]
    g["NTAB"] = g["RANKS"] * g["PERP"]       # padded kv-table rows
    g["NI"] = g["T"] * 128                   # edge slots per dst block
    assert g["NTAB"] < 32768                 # int16 gather indices
    wfull_b = WTOT + (3 * D + OUT_CH) * 4    # weights + bias rows
    wfull_b += -wfull_b % g["RANKS"]
    g["WSH"] = wfull_b // g["RANKS"]         # weight shard bytes per rank
    ic = 128 * g["NB"] * (g["NI"] // 16) * 2
    sizes = dict(
        x_own=g["PERP"] * D_IN * 2,
        wsh=g["WSH"],
        kvidx=ic,                            # wrapped int16, [128, NB*NI/16]
        dqidx=ic,                            # local dst idx for the q gather
        dstrel=128 * g["NB"] * g["T"] * 2,   # bf16 dst-within-block, [128, NB*T]
        iota=128 * 128 * 2,
    )
    offs, off = {}, 0
    for k, s in sizes.items():
        offs[k] = off
        off += s
    g["BLOB_OFFS"], g["BLOB_BYTES"] = offs, off
    return g


# ----------------------------------------------------------------------------
# Program builder
# ----------------------------------------------------------------------------

def build_program(cfg):
    import concourse.bass as bass
    import concourse.mybir as mybir
    import concourse.tile as tile
    from concourse import bacc
    from concourse.masks import make_identity

    g = derive(cfg)
    NB, T, NI, NTAB, PERP, WSH = (g["NB"], g["T"], g["NI"], g["NTAB"], g["PERP"],
                                  g["WSH"])
    NRANKS = g["RANKS"]
    OFFS = g["BLOB_OFFS"]
    F32, BF16, I16, U8 = (mybir.dt.float32, mybir.dt.bfloat16, mybir.dt.int16,
                          mybir.dt.uint8)
    AF = mybir.ActivationFunctionType
    OP = mybir.AluOpType
    SCALE = 1.0 / math.sqrt(C)

    nc = bacc.Bacc("TRN2", target_bir_lowering=False, debug=False,
                   num_devices=NRANKS)

    blob = nc.dram_tensor("blob", [g["BLOB_BYTES"]], U8, kind="ExternalInput").ap()
    out_t = nc.dram_tensor("out", [PERP, OUT_CH], BF16, kind="ExternalOutput").ap()

    def bv(key, nbytes):
        o = OFFS[key]
        return blob[o:o + nbytes]

    rg = [list(range(NRANKS))]

    with tile.TileContext(nc) as tc:
        with (
            tc.tile_pool(name="dram", bufs=1, space="DRAM") as dpool,
            tc.tile_pool(name="const", bufs=1) as cpool,
        ):
            kv0_own = dpool.tile([PERP, 2 * D], BF16, tag="kv0o")
            kv0_t = dpool.tile([NTAB, 2 * D], BF16, tag="kv0", addr_space="Shared")
            kv1_own = dpool.tile([PERP, 2 * D], BF16, tag="kv1o")
            kv1_t = dpool.tile([NTAB, 2 * D], BF16, tag="kv1", addr_space="Shared")
            q0_t = dpool.tile([PERP + 128, D], BF16, tag="q0")
            q1_t = dpool.tile([PERP + 128, D], BF16, tag="q1")
            sk0_t = dpool.tile([PERP, D], F32, tag="sk0")
            sk1_t = dpool.tile([PERP, OUT_CH], F32, tag="sk1")
            wb = dpool.tile([WSH], U8, tag="wb")
            wfull = dpool.tile([NRANKS * WSH], U8, tag="wfull", addr_space="Shared")

            nc.sync.dma_start(out=wb[:], in_=bv("wsh", WSH))
            nc.gpsimd.collective_compute(
                "AllGather", OP.bypass, replica_groups=rg,
                ins=[wb.opt()], outs=[wfull.opt()])

            def wview(i, shape_str, **kw):
                v = wfull[int(W_OFFS[i]):int(W_OFFS[i]) + W_SIZES[i] * 2]
                return v.bitcast(BF16).rearrange(shape_str, **kw)

            w0kv_s = cpool.tile([128, 2 * D], BF16, tag="w0kv")
            nc.sync.dma_start(out=w0kv_s[:], in_=wview(0, "(p d) -> p d", p=128))
            w0q_s = cpool.tile([128, D], BF16, tag="w0q")
            nc.sync.dma_start(out=w0q_s[:], in_=wview(1, "(p d) -> p d", p=128))
            w0s_s = cpool.tile([128, D], BF16, tag="w0s")
            nc.sync.dma_start(out=w0s_s[:], in_=wview(2, "(p d) -> p d", p=128))
            w1kv_s = cpool.tile([128, 4, 2 * D], BF16, tag="w1kv")
            nc.sync.dma_start(out=w1kv_s[:], in_=wview(3, "(p s d) -> p s d", p=128, s=4))
            w1q_s = cpool.tile([128, 4, D], BF16, tag="w1q")
            nc.sync.dma_start(out=w1q_s[:], in_=wview(4, "(p s d) -> p s d", p=128, s=4))
            w1s_s = cpool.tile([128, 4, OUT_CH], BF16, tag="w1s")
            nc.sync.dma_start(out=w1s_s[:], in_=wview(5, "(p s d) -> p s d", p=128, s=4))

            ident_s = cpool.tile([128, 128], BF16, tag="ident")
            make_identity(nc, ident_s[:])
            iota_s = cpool.tile([128, 128], BF16, tag="iota")
            nc.sync.dma_start(out=iota_s[:],
                              in_=bv("iota", 128 * 128 * 2).bitcast(BF16)
                              .rearrange("(p d) -> p d", p=128))

            # bias rows live in the last 6656 bytes of the weights blob (see host_prep)
            ones_s = cpool.tile([1, 128], F32, tag="ones")
            nc.vector.memset(ones_s[:], 1.0)
            brow_s = cpool.tile([1, 3 * D + OUT_CH], F32, tag="brow")
            boff = int(W_OFFS[5]) + W_SIZES[5] * 2
            nc.sync.dma_start(out=brow_s[:],
                              in_=wfull[boff:boff + (3 * D + OUT_CH) * 4]
                              .bitcast(F32).rearrange("(o d) -> o d", o=1))
            b0q_s = cpool.tile([128, D], F32, tag="b0q")
            c0_s = cpool.tile([128, D], F32, tag="c0")
            b1q_s = cpool.tile([128, D], F32, tag="b1q")
            c1_s = cpool.tile([128, OUT_CH], F32, tag="c1")

            with (
                tc.tile_pool(name="work", bufs=1) as pool,
                tc.tile_pool(name="roll", bufs=3) as rpool,
                tc.tile_pool(name="psum", bufs=1, space="PSUM") as pp,
            ):
                for bi, (btile, w) in enumerate(
                        [(b0q_s, D), (c0_s, D), (b1q_s, D), (c1_s, OUT_CH)]):
                    pb = pp.tile([128, D], F32, tag="pa")
                    nc.tensor.matmul(pb[:, :w], ones_s[:],
                                     brow_s[:, bi * D:bi * D + w], start=True, stop=True)
                    nc.scalar.activation(btile[:], pb[:, :w], AF.Copy)

                # ---------------- P0: layer-0 projections (own rows only) -------
                x_own_v = bv("x_own", PERP * D_IN * 2).bitcast(BF16).rearrange(
                    "(n d) -> n d", d=D_IN)
                for btg in range(NB // 4):
                    xtg = rpool.tile([128, 512], BF16, tag="xtg")
                    nc.sync.dma_start(out=xtg[:],
                                      in_=x_own_v[btg * 512:(btg + 1) * 512, :],
                                      transpose=True)
                    for sub in range(4):
                        bt = btg * 4 + sub
                        lhsT = xtg[:, sub * 128:(sub + 1) * 128]
                        pk = pp.tile([128, D], F32, tag="pk")
                        pv = pp.tile([128, D], F32, tag="pv")
                        nc.tensor.matmul(pk[:], lhsT, w0kv_s[:, 0:D], start=True, stop=True)
                        nc.tensor.matmul(pv[:], lhsT, w0kv_s[:, D:2 * D], start=True, stop=True)
                        kvb = rpool.tile([128, 2 * D], BF16, tag="kvb")
                        nc.scalar.activation(kvb[:, 0:D], pk[:], AF.Copy)
                        nc.vector.tensor_copy(out=kvb[:, D:2 * D], in_=pv[:])
                        nc.sync.dma_start(out=kv0_own[bt * 128:(bt + 1) * 128, :], in_=kvb[:])
                        pq = pp.tile([128, D], F32, tag="pk")
                        nc.tensor.matmul(pq[:], lhsT, w0q_s[:], start=True, stop=True)
                        qb = rpool.tile([128, D], BF16, tag="kvb")
                        nc.vector.tensor_tensor(out=qb[:], in0=pq[:], in1=b0q_s[:], op=OP.add)
                        nc.sync.dma_start(out=q0_t[bt * 128:(bt + 1) * 128, :], in_=qb[:])
                        ps = pp.tile([128, D], F32, tag="pv")
                        nc.tensor.matmul(ps[:], lhsT, w0s_s[:], start=True, stop=True)
                        skb = rpool.tile([128, D], F32, tag="skb")
                        nc.vector.tensor_tensor(out=skb[:], in0=ps[:], in1=c0_s[:], op=OP.add)
                        nc.sync.dma_start(out=sk0_t[bt * 128:(bt + 1) * 128, :], in_=skb[:])
                nc.gpsimd.collective_compute(
                    "AllGather", OP.bypass, replica_groups=rg,
                    ins=[kv0_own.opt()], outs=[kv0_t.opt()])

                kvidx_v = bv("kvidx", 128 * NB * (NI // 16) * 2).bitcast(I16).rearrange(
                    "(p s) -> p s", p=128)
                dqidx_v = bv("dqidx", 128 * NB * (NI // 16) * 2).bitcast(I16).rearrange(
                    "(p s) -> p s", p=128)
                dstrel_v = bv("dstrel", 128 * NB * T * 2).bitcast(BF16).rearrange(
                    "(p s) -> p s", p=128)

                # ---------------- edge phase (both layers) ----------------
                def edge_layer(layer):
                    kv_tab = kv0_t if layer == 0 else kv1_t
                    q_tab = q0_t if layer == 0 else q1_t
                    ikv_all = cpool.tile([128, NB * (NI // 16)], I16, tag=f"ikv{layer}")
                    nc.sync.dma_start(out=ikv_all[:], in_=kvidx_v)
                    idq_all = cpool.tile([128, NB * (NI // 16)], I16, tag=f"idq{layer}")
                    nc.sync.dma_start(out=idq_all[:], in_=dqidx_v)
                    dr_all = cpool.tile([128, NB * T], BF16, tag=f"dr{layer}")
                    nc.sync.dma_start(out=dr_all[:], in_=dstrel_v)
                    for b in range(NB):
                        i0 = b * (NI // 16)
                        kvt = pool.tile([128, T, 2 * D], BF16, tag="kvt")
                        qt = pool.tile([128, T, D], BF16, tag="qt")
                        for g0 in range(0, T, 8):
                            gt = min(8, T - g0)
                            ni = gt * 128
                            nc.gpsimd.dma_gather(
                                kvt[:, g0:g0 + gt, :], kv_tab[:],
                                ikv_all[:, i0 + g0 * 8:i0 + g0 * 8 + ni // 16],
                                ni, ni, elem_size=2 * D)
                            nc.gpsimd.dma_gather(
                                qt[:, g0:g0 + gt, :], q_tab[:],
                                idq_all[:, i0 + g0 * 8:i0 + g0 * 8 + ni // 16],
                                ni, ni, elem_size=D)
                        qk = pool.tile([128, T, D], BF16, tag="qk")
                        nc.vector.tensor_tensor(out=qk[:], in0=qt[:],
                                                in1=kvt[:, :, 0:D], op=OP.mult)
                        alpha = rpool.tile([128, 4 * T], F32, tag="alpha")
                        nc.vector.tensor_reduce(
                            out=alpha[:].rearrange("p (t h) -> p t h", h=H),
                            in_=qk[:].rearrange("p t (h c) -> p t h c", c=C),
                            axis=mybir.AxisListType.X, op=OP.add)
                        ae = rpool.tile([128, 4 * T], F32, tag="ae")
                        nc.scalar.activation(ae[:], alpha[:], AF.Exp, scale=SCALE)
                        aeb = rpool.tile([128, 4 * T], BF16, tag="aeb")
                        nc.vector.tensor_copy(out=aeb[:], in_=ae[:])
                        ct = pool.tile([128, T, D], BF16, tag="ct")
                        nc.vector.tensor_tensor(
                            out=ct[:].rearrange("p t (h c) -> p t h c", c=C),
                            in0=kvt[:, :, D:2 * D].rearrange("p t (h c) -> p t h c", c=C),
                            in1=aeb[:].rearrange("p (t h o) -> p t h o", h=H, o=1)
                            .to_broadcast([128, T, H, C]),
                            op=OP.mult)
                        Ma = pool.tile([128, T, 128], BF16, tag="Ma")
                        nc.vector.tensor_tensor(
                            out=Ma[:],
                            in0=dr_all[:, b * T:(b + 1) * T]
                            .rearrange("p (t o) -> p t o", o=1)
                            .to_broadcast([128, T, 128]),
                            in1=iota_s[:].rearrange("p (o d) -> p o d", o=1)
                            .to_broadcast([128, T, 128]),
                            op=OP.is_equal)
                        po = pp.tile([128, D], F32, tag="po")
                        ps = pp.tile([128, 4], F32, tag="ps")
                        for j in range(T):
                            Mj = Ma[:, j:j + 1, :].rearrange("p o d -> p (o d)")
                            nc.tensor.matmul(ps[:], Mj, aeb[:, 4 * j:4 * j + 4],
                                             start=(j == 0), stop=(j == T - 1))
                            nc.tensor.matmul(po[:], Mj,
                                             ct[:, j:j + 1, :].rearrange("p o d -> p (o d)"),
                                             start=(j == 0), stop=(j == T - 1))

                        # ---- block finalize ----
                        rsl = slice(b * 128, (b + 1) * 128)
                        se = rpool.tile([128, H], F32, tag="se")
                        nc.vector.tensor_scalar_add(se[:], ps[:], 1e-30)
                        iv = rpool.tile([128, H], F32, tag="iv")
                        nc.vector.reciprocal(out=iv[:], in_=se[:])
                        if layer == 0:
                            hb = rpool.tile([128, D], F32, tag="hb")
                            nc.vector.tensor_tensor(
                                out=hb[:].rearrange("p (h c) -> p h c", c=C),
                                in0=po[:].rearrange("p (h c) -> p h c", c=C),
                                in1=iv[:].to_broadcast([128, H, C]),
                                op=OP.mult)
                            skb = rpool.tile([128, D], F32, tag="skb")
                            nc.sync.dma_start(out=skb[:], in_=sk0_t[rsl, :])
                            nc.vector.tensor_tensor(out=hb[:], in0=hb[:], in1=skb[:], op=OP.add)
                            hbb = rpool.tile([128, D], BF16, tag="hbb")
                            nc.scalar.activation(hbb[:], hb[:], AF.Relu)
                            hTall = rpool.tile([128, 4, 128], BF16, tag="hTall")
                            for sg in range(4):
                                pt = pp.tile([128, 128], BF16, tag="pa")
                                nc.tensor.transpose(pt[:], hbb[:, sg * 128:(sg + 1) * 128],
                                                    ident_s[:])
                                nc.scalar.activation(
                                    hTall[:, sg:sg + 1, :].rearrange("p o d -> p (o d)"),
                                    pt[:], AF.Copy)
                            pq = pp.tile([128, D], F32, tag="pq")
                            for sg in range(4):
                                nc.tensor.matmul(
                                    pq[:], hTall[:, sg:sg + 1, :].rearrange("p o d -> p (o d)"),
                                    w1q_s[:, sg:sg + 1, :].rearrange("p o d -> p (o d)"),
                                    start=(sg == 0), stop=(sg == 3))
                            qb = rpool.tile([128, D], BF16, tag="qb1")
                            nc.vector.tensor_tensor(out=qb[:], in0=pq[:], in1=b1q_s[:], op=OP.add)
                            nc.sync.dma_start(out=q1_t[rsl, :], in_=qb[:])
                            pkk = pp.tile([128, D], F32, tag="pkk")
                            pvv = pp.tile([128, D], F32, tag="pvv")
                            for sg in range(4):
                                lhsT = hTall[:, sg:sg + 1, :].rearrange("p o d -> p (o d)")
                                nc.tensor.matmul(pkk[:], lhsT,
                                                 w1kv_s[:, sg:sg + 1, 0:D].rearrange("p o d -> p (o d)"),
                                                 start=(sg == 0), stop=(sg == 3))
                                nc.tensor.matmul(pvv[:], lhsT,
                                                 w1kv_s[:, sg:sg + 1, D:2 * D].rearrange("p o d -> p (o d)"),
                                                 start=(sg == 0), stop=(sg == 3))
                            kvb = rpool.tile([128, 2 * D], BF16, tag="kvb1")
                            nc.scalar.activation(kvb[:, 0:D], pkk[:], AF.Copy)
                            nc.scalar.activation(kvb[:, D:2 * D], pvv[:], AF.Copy)
                            nc.sync.dma_start(out=kv1_own[rsl, :], in_=kvb[:])
                            psk = pp.tile([128, OUT_CH], F32, tag="pq")
                            for sg in range(4):
                                nc.tensor.matmul(
                                    psk[:], hTall[:, sg:sg + 1, :].rearrange("p o d -> p (o d)"),
                                    w1s_s[:, sg:sg + 1, :].rearrange("p o d -> p (o d)"),
                                    start=(sg == 0), stop=(sg == 3))
                            s1b = rpool.tile([128, OUT_CH], F32, tag="s1b")
                            nc.vector.tensor_tensor(out=s1b[:], in0=psk[:], in1=c1_s[:], op=OP.add)
                            nc.sync.dma_start(out=sk1_t[rsl, :], in_=s1b[:])
                        else:
                            iv4 = rpool.tile([128, H], F32, tag="iv4")
                            nc.vector.tensor_scalar_mul(iv4[:], iv[:], 1.0 / H)
                            nrm = rpool.tile([128, D], F32, tag="nrm")
                            nc.vector.tensor_tensor(
                                out=nrm[:].rearrange("p (h c) -> p h c", c=C),
                                in0=po[:].rearrange("p (h c) -> p h c", c=C),
                                in1=iv4[:].to_broadcast([128, H, C]),
                                op=OP.mult)
                            mn = rpool.tile([128, OUT_CH], F32, tag="mn")
                            nc.vector.tensor_reduce(
                                out=mn[:], in_=nrm[:].rearrange("p (h c) -> p c h", c=C),
                                axis=mybir.AxisListType.X, op=OP.add)
                            skb = rpool.tile([128, OUT_CH], F32, tag="skb1")
                            nc.sync.dma_start(out=skb[:], in_=sk1_t[rsl, :])
                            ob = rpool.tile([128, OUT_CH], BF16, tag="ob")
                            nc.vector.tensor_tensor(out=ob[:], in0=mn[:], in1=skb[:], op=OP.add)
                            nc.sync.dma_start(out=out_t[rsl, :], in_=ob[:])
                    if layer == 0:
                        nc.gpsimd.collective_compute(
                            "AllGather", OP.bypass, replica_groups=rg,
                            ins=[kv1_own.opt()], outs=[kv1_t.opt()])

                edge_layer(0)
                edge_layer(1)


    nc.compile()
    return nc


# ----------------------------------------------------------------------------
# Host-side preparation
# ----------------------------------------------------------------------------

def host_prep(cfg, x, edge_index,
              Wq0, bq0, Wk0, bk0, Wv0, bv0, Ws0, bs0,
              Wq1, bq1, Wk1, bk1, Wv1, bv1, Ws1, bs1):
    g = derive(cfg)
    NRANKS, NB, T, NI, PER, PERP = (g["RANKS"], g["NB"], g["T"], g["NI"],
                                    g["PER"], g["PERP"])
    OFFS, TOT = g["BLOB_OFFS"], g["BLOB_BYTES"]

    x = np.asarray(x, np.float32)
    src = np.asarray(edge_index[0], np.int64)
    dst = np.asarray(edge_index[1], np.int64)
    nprime = (src // PER) * PERP + (src % PER)   # src id in padded kv tables

    w0kv = np.concatenate([np.asarray(Wk0, np.float32).astype(BF),
                           np.asarray(Wv0, np.float32).astype(BF)], 1)
    bias = np.concatenate([
        np.asarray(bq0, np.float32),
        (np.asarray(bs0) + np.asarray(bv0)).astype(np.float32),
        np.asarray(bq1, np.float32),
        (np.asarray(bs1) + np.asarray(bv1, np.float32).reshape(H, OUT_CH).mean(0))
        .astype(np.float32),
    ]).view(np.uint8)
    wblob = np.concatenate([
        w0kv.reshape(-1).view(np.uint8),
        np.asarray(Wq0, np.float32).astype(BF).reshape(-1).view(np.uint8),
        np.asarray(Ws0, np.float32).astype(BF).reshape(-1).view(np.uint8),
        np.ascontiguousarray(
            np.concatenate([Wk1, Wv1], 1).astype(BF).reshape(4, 128, 2 * D)
            .transpose(1, 0, 2)).reshape(-1).view(np.uint8),
        np.ascontiguousarray(np.asarray(Wq1, np.float32).astype(BF)
                             .reshape(4, 128, D).transpose(1, 0, 2)).reshape(-1).view(np.uint8),
        np.ascontiguousarray(np.asarray(Ws1, np.float32).astype(BF)
                             .reshape(4, 128, OUT_CH).transpose(1, 0, 2)).reshape(-1).view(np.uint8),
        bias,
    ])
    assert wblob.nbytes == WTOT + bias.nbytes
    wpad = NRANKS * g["WSH"] - wblob.nbytes
    wblob = np.concatenate([wblob, np.zeros(wpad, np.uint8)])

    def wrap_idx(arr):  # [NB, NI] int -> [128, NB*NI//16] int16 (16-wrap, 8x replicated)
        a = arr.reshape(NB, NI // 16, 16).transpose(0, 2, 1)   # [NB, 16, NI//16]
        a = np.tile(a, (1, 8, 1))                               # [NB, 128, NI//16]
        return np.ascontiguousarray(a.transpose(1, 0, 2)
                                    .reshape(128, NB * (NI // 16)).astype(np.int16))

    iota = np.tile(np.arange(128).astype(BF)[None], (128, 1)).view(np.uint8).reshape(-1)

    in_maps = []
    for r in range(NRANKS):
        lo, hi = r * PER, (r + 1) * PER
        m = (dst >= lo) & (dst < hi)
        es, ed, npr = src[m], dst[m] - lo, nprime[m]
        blk = ed // 128
        order = np.argsort(blk, kind="stable")
        ed, npr, blk = ed[order], npr[order], blk[order]
        cnt = np.bincount(blk, minlength=NB)
        assert cnt.max() <= NI, f"block overflow: {cnt.max()} > {NI}"
        kvi = np.zeros((NB, NI), np.int64)
        dqi = np.zeros((NB, NI), np.int64)
        drl = np.full((NB, NI), -1.0, np.float32)
        pos = 0
        for b in range(NB):
            nb = cnt[b]
            sl = slice(pos, pos + nb)
            kvi[b, :nb] = npr[sl]
            dqi[b, :nb] = ed[sl]
            drl[b, :nb] = (ed[sl] % 128).astype(np.float32)
            pos += nb
        xo = np.zeros((PERP, D_IN), BF)
        xo[:PER] = x[lo:hi]
        blob = np.empty(TOT, np.uint8)
        blob[OFFS["x_own"]:OFFS["x_own"] + xo.nbytes] = xo.view(np.uint8).reshape(-1)
        blob[OFFS["wsh"]:OFFS["wsh"] + g["WSH"]] = wblob[r * g["WSH"]:(r + 1) * g["WSH"]]
        kb = wrap_idx(kvi).view(np.uint8).reshape(-1)
        blob[OFFS["kvidx"]:OFFS["kvidx"] + kb.nbytes] = kb
        qb = wrap_idx(dqi).view(np.uint8).reshape(-1)
        blob[OFFS["dqidx"]:OFFS["dqidx"] + qb.nbytes] = qb
        db = np.ascontiguousarray(drl.reshape(NB, T, 128).transpose(2, 0, 1)
                                  .reshape(128, NB * T).astype(BF)).view(np.uint8).reshape(-1)
        blob[OFFS["dstrel"]:OFFS["dstrel"] + db.nbytes] = db
        blob[OFFS["iota"]:OFFS["iota"] + iota.nbytes] = iota
        in_maps.append(dict(blob=blob))
    return in_maps


# ----------------------------------------------------------------------------
# Entry point
# ----------------------------------------------------------------------------

_CACHE = {}


def _get_program():
    if "nc" not in _CACHE:
        _CACHE["nc"] = build_program(FULL_CFG)
    return _CACHE["nc"]


def run_on_hw(nc, in_maps, cfg, trace=False):
    from concourse import bass_utils
    g = derive(cfg)
    res = bass_utils.run_bass_kernel_spmd(
        nc, in_maps, core_ids=list(range(g["RANKS"])), trace=trace)
    outs = [np.asarray(res.results[r]["out"][:g["PER"]], np.float32)
            for r in range(g["RANKS"])]
    return np.concatenate(outs, 0), res


def kernel(x, edge_index,
           Wq0, bq0, Wk0, bk0, Wv0, bv0, Ws0, bs0,
           Wq1, bq1, Wk1, bk1, Wv1, bv1, Ws1, bs1):
    nc = _get_program()
    in_maps = host_prep(FULL_CFG, x, edge_index,
                        Wq0, bq0, Wk0, bk0, Wv0, bv0, Ws0, bs0,
                        Wq1, bq1, Wk1, bk1, Wv1, bv1, Ws1, bs1)
    out, _ = run_on_hw(nc, in_maps, FULL_CFG)
    return out
